# revision 2
# baseline (speedup 1.0000x reference)
"""Trainium2 Bass kernel for nn_MessagePassing (gnn_message_passing).

Reference computation (2 steps):
    h    = relu(cur @ mW1 + mb1)                      # per-module MLP layer 1
    msg  = h @ mW2 + mb2                              # per-module MLP layer 2
    rec  = einsum('mn,bnd->bmd', C, msg) * w[:,:,None]
    g    = relu(concat([cur, rec], -1) @ aW1 + ab1)
    cur  = cur + g @ aW2 + ab2

Strategy (data-parallel over 8 NeuronCores, 8192 batch rows each):
  * T-layout on chip: SBUF tiles are [128 features (partitions), cols] where
    a column is one (b, m) row of the flattened [B*M, 128] stream. Every
    per-module linear layer is one weights-stationary bf16 matmul
    (out = W.T @ x_T) streaming 512 columns per instruction; PSUM fp32.
  * Algebraic refactor (host-side): row scaling by w commutes with
    right-matmuls and the 8x8 mix commutes with feature transforms:
        rec_contrib = (w ⊙ mix(h)) @ Q + s ⊗ qb
    with Q = mW2 @ aW1_bot, qb = mb2 @ aW1_bot, s = w * rowsum(C). This
    eliminates the mW2 pass. The rank-1 s⊗qb term is folded into the
    aW1_top operand: with y = solve(aW1_top.T, qb), (cur + y⊗s) @ aW1_top
    = cur @ aW1_top + s ⊗ qb — the host ships xa = x + y⊗s alongside xb,
    and step-1 rebuilds c1a = xa + u1 with one cheap bf16 DVE op. No
    rank-1 matmul on the PE at all (5 streams/group-step, not 6).
  * The module mix runs on the PE as ONE weights-stationary matmul with
    W_mix = kron(I_16, C.T) (block-diagonal over 16 b-groups x 8 modules).
    Rows<->partitions movement uses the DMA XBAR sub-tiled transpose
    ([128,(t,d)] <-> [d,(t,128)]), batched 4 groups (2048 cols) per DMA
    instruction. ab2 is deferred to the host; the bf16 residual stream
    (u1b) makes the step-1 operand rebuilds all-SBUF-bf16 (DVE 4x mode).
  * DRAM I/O is quad-blocked ([nq, 128, 2048] contiguous 1MB blocks).
  * Engine balance per group-step: PE 5 matmuls; ACT h-relu evac (3/4) +
    g-relu + u1b; DVE h-relu (1/4) + w-scale + c1b/c1a/onew; SP the
    transposes (one queue; XBAR state is shared, never split transposes
    across queues); GPSIMD all DRAM I/O triggers.
  * Emission is software-pipelined over super-groups of quads: all
    front-half work (h matmul, relu, trA, mix, w-scale, trB) for a quad is
    emitted before its back-half (aW1 accumulate, relu, aW2, residual),
    so the PE's in-order stream never waits on the transpose round-trip.
"""

import os
import sys

import numpy as np

try:
    import concourse.bass as bass
except ImportError:  # harness runs kernel.py from a bare directory
    sys.path.insert(0, "/opt/trn_rl_repo")
    import concourse.bass as bass

import ml_dtypes
import concourse.bacc as bacc
import concourse.mybir as mybir
from concourse.tile import TileContext

BF16 = ml_dtypes.bfloat16
D = 128
M = 8
GRP = 512
QB = 4                  # groups per quad (DMA/transpose batch)
QCOLS = QB * GRP        # 2048
SGQ = 5                 # quads per super-group (software pipeline depth)
NCORES = 8
NSTEPS = 2

_nc_cache = {}


def build_nc(cols):
    """Build (and cache) the per-core Bass program for a `cols`-wide shard."""
    if cols in _nc_cache:
        return _nc_cache[cols]
    assert cols % QCOLS == 0
    ng = cols // GRP
    nq = cols // QCOLS

    f32 = mybir.dt.float32
    bf = mybir.dt.bfloat16
    relu = mybir.ActivationFunctionType.Relu
    add = mybir.AluOpType.add
    mult = mybir.AluOpType.mult
    amax = mybir.AluOpType.max

    nc = bacc.Bacc(trn_type="TRN2")
    xb_d = nc.declare_dram_parameter("xb", [nq, D, QCOLS], bf, isOutput=False)
    xa_d = nc.declare_dram_parameter("xa", [nq, D, QCOLS], bf, isOutput=False)
    wcol_d = nc.declare_dram_parameter("wcol", [D, 4 * ng], f32, isOutput=False)
    wm1_d = nc.declare_dram_parameter("wm1", [D, D], bf, isOutput=False)
    wmx_d = nc.declare_dram_parameter("wmx", [D, D], bf, isOutput=False)
    wq_d = nc.declare_dram_parameter("wq", [D, D], bf, isOutput=False)
    wa1t_d = nc.declare_dram_parameter("wa1t", [D, D], bf, isOutput=False)
    wa2_d = nc.declare_dram_parameter("wa2", [D, D], bf, isOutput=False)
    mb1_d = nc.declare_dram_parameter("mb1", [D, 1], f32, isOutput=False)
    ab1_d = nc.declare_dram_parameter("ab1", [D, 1], f32, isOutput=False)
    ab2_d = nc.declare_dram_parameter("ab2", [D, 1], f32, isOutput=False)
    out_d = nc.declare_dram_parameter("out", [nq, D, QCOLS], bf, isOutput=True)

    nsq = (nq + SGQ - 1) // SGQ

    with TileContext(nc) as tc:
        with (
            tc.tile_pool(name="consts", bufs=1) as cp,
            tc.tile_pool(name="work", bufs=2) as wp,
            tc.tile_pool(name="pipe", bufs=SGQ + 1) as fp,
            tc.tile_pool(name="psum", bufs=2, space="PSUM") as pp,
        ):
            w_m1 = cp.tile_from(forced_dma_engine=mybir.EngineType.Pool, ap=wm1_d[:, :])
            w_mx = cp.tile_from(forced_dma_engine=mybir.EngineType.Pool, ap=wmx_d[:, :])
            w_q = cp.tile_from(forced_dma_engine=mybir.EngineType.Pool, ap=wq_d[:, :])
            w_a1t = cp.tile_from(forced_dma_engine=mybir.EngineType.Pool, ap=wa1t_d[:, :])
            w_a2 = cp.tile_from(forced_dma_engine=mybir.EngineType.Pool, ap=wa2_d[:, :])
            wcol = cp.tile_from(forced_dma_engine=mybir.EngineType.Pool, ap=wcol_d[:, :])
            mb1 = cp.tile_from(forced_dma_engine=mybir.EngineType.Pool, ap=mb1_d[:, :])
            ab1 = cp.tile_from(forced_dma_engine=mybir.EngineType.Pool, ap=ab1_d[:, :])
            ab2 = cp.tile_from(forced_dma_engine=mybir.EngineType.Pool, ap=ab2_d[:, :])

            # live tensors per quad within a super-group
            xb = [None] * SGQ     # bf16 input (step-1 h operand, clean)
            xa = [None] * SGQ     # bf16 input with y⊗s fold (a1t operand)
            u1b = [None] * SGQ    # bf16 upd1 + ab2 (step-2 residual base)
            c1b = [None] * SGQ    # bf16 step-2 h operand (xb + u1b)
            c1a = [None] * SGQ    # bf16 step-2 a1t operand (xa + u1b)
            smixT = [None] * SGQ  # bf16 mix output, T-layout

            hRs = [None] * SGQ

            def frontA(q, step, i):
                if step == 0:
                    xb[i] = fp.tile([D, QCOLS], bf, tag="xb", name=f"xb{i}")
                    nc.gpsimd.dma_start(xb[i][:], xb_d[q])
                    xa[i] = fp.tile([D, QCOLS], bf, tag="xa", name=f"xa{i}")
                    nc.gpsimd.dma_start(xa[i][:], xa_d[q])
                cur_b = xb[i] if step == 0 else c1b[i]
                h = wp.tile([D, QCOLS], bf, tag="h")
                for j in range(QB):
                    cs = slice(j * GRP, (j + 1) * GRP)
                    hp = pp.tile([D, GRP], f32, tag="hp")
                    nc.tensor.matmul(
                        hp[:], w_m1[:], cur_b[:, cs], start=True, stop=True
                    )
                    if j % 4 == 3:
                        nc.vector.tensor_scalar(
                            h[:, cs], hp[:], mb1[:], 0.0, add, amax
                        )
                    else:
                        nc.scalar.activation(h[:, cs], hp[:], relu, bias=mb1[:])
                # NB: ALL transposes ride ONE DMA queue (nc.sync) carrying
                # no DMACopy traffic: mixing kinds on a queue, or running
                # transposes on two queues concurrently, corrupts data via
                # the shared XBAR state. Copies go via nc.gpsimd only.
                hRs[i] = wp.tile([D, QB * 4, D], bf, tag="hR", bufs=3,
                                 name=f"hR{i}")
                nc.sync.dma_start_transpose(hRs[i][:], h[:])

            def frontB(q, step, i):
                hR = hRs[i]
                smix = wp.tile([D, QCOLS], bf, tag="smix")
                for j in range(QB):
                    g = q * QB + j
                    cs = slice(j * GRP, (j + 1) * GRP)
                    mixp = pp.tile([D, GRP], f32, tag="mixp")
                    nc.tensor.matmul(
                        mixp[:], w_mx[:], hR[:, j * 4 : (j + 1) * 4, :],
                        start=True, stop=True,
                    )
                    # w-scale evac: one op, 0-stride broadcast of 4 w-columns
                    nc.vector.tensor_tensor(
                        smix[:, cs].rearrange("a (b c) -> a b c", b=4),
                        mixp[:].rearrange("a (b c) -> a b c", b=4),
                        wcol[:, 4 * g : 4 * g + 4].broadcast_to((D, 4, D)),
                        mult,
                    )
                smixT[i] = fp.tile([D, QB * 4, D], bf, tag="smixT", name=f"sT{i}")
                half = QCOLS // 2
                nc.sync.dma_start_transpose(
                    smixT[i][:, : QB * 2, :], smix[:, :half]
                )
                nc.sync.dma_start_transpose(
                    smixT[i][:, QB * 2 :, :], smix[:, half:]
                )

            def back(q, step, i):
                a_b = xa[i] if step == 0 else c1a[i]
                if step == NSTEPS - 1:
                    onew = wp.tile([D, QCOLS], bf, tag="onew", bufs=3, name="onew")
                else:
                    u1b[i] = fp.tile([D, QCOLS], bf, tag="u1b", name=f"u1b{i}")
                    c1b[i] = fp.tile([D, QCOLS], bf, tag="c1b", name=f"c1b{i}")
                    c1a[i] = fp.tile([D, QCOLS], bf, tag="c1a", name=f"c1a{i}")
                for j in range(QB):
                    cs = slice(j * GRP, (j + 1) * GRP)
                    gp = pp.tile([D, GRP], f32, tag="gp")
                    nc.tensor.matmul(
                        gp[:], w_q[:], smixT[i][:, j * 4 : (j + 1) * 4, :],
                        start=True, stop=False,
                    )
                    nc.tensor.matmul(
                        gp[:], w_a1t[:], a_b[:, cs], start=False, stop=True
                    )
                    gt = wp.tile([D, GRP], bf, tag="gt")
                    nc.scalar.activation(gt[:], gp[:], relu, bias=ab1[:])
                    up = pp.tile([D, GRP], f32, tag="up")
                    nc.tensor.matmul(
                        up[:], w_a2[:], gt[:], start=True, stop=True
                    )
                    if step < NSTEPS - 1:
                        # u1b = up1 + ab2 (bf16, step-2 residual base)
                        nc.scalar.activation(
                            u1b[i][:, cs], up[:],
                            mybir.ActivationFunctionType.Identity, bias=ab2[:],
                        )
                        # step-2 operands: all-bf16 SBUF adds (DVE 4x mode)
                        nc.vector.tensor_tensor(
                            c1b[i][:, cs], u1b[i][:, cs], xb[i][:, cs], add
                        )
                        nc.vector.tensor_tensor(
                            c1a[i][:, cs], u1b[i][:, cs], xa[i][:, cs], add
                        )
                    else:
                        # device out = up1' + up2; host adds x + ab2
                        nc.vector.tensor_tensor(
                            onew[:, cs], up[:], u1b[i][:, cs], add
                        )
                if step == NSTEPS - 1:
                    nc.gpsimd.dma_start(out_d[q], onew[:])

            for sq in range(nsq):
                qs = [q for q in range(sq * SGQ, min((sq + 1) * SGQ, nq))]
                for step in range(NSTEPS):
                    nq_s = len(qs)
                    lag = 2 if nq_s > 2 else 1
                    for k in range(nq_s + lag):
                        if k < nq_s:
                            frontA(qs[k], step, k)
                        if k >= lag:
                            frontB(qs[k - lag], step, k - lag)
                    for q in qs:
                        back(q, step, q - sq * SGQ)

    nc.compile()
    _nc_cache[cols] = nc
    return nc


def host_prep(module_states, connection_matrix, module_weights,
              mW1, mb1, mW2, mb2, aW1, ab1, aW2, ab2, ncores=NCORES):
    """Shard + precompute all host-side tensors. Returns (cols, in_maps)."""
    ms = np.asarray(module_states, np.float32)
    C = np.asarray(connection_matrix, np.float32)
    w = np.asarray(module_weights, np.float32)
    mW1 = np.asarray(mW1, np.float32)
    mb1 = np.asarray(mb1, np.float32)
    mW2 = np.asarray(mW2, np.float32)
    mb2 = np.asarray(mb2, np.float32)
    aW1 = np.asarray(aW1, np.float32)
    ab1 = np.asarray(ab1, np.float32)
    aW2 = np.asarray(aW2, np.float32)
    ab2 = np.asarray(ab2, np.float32)

    B = ms.shape[0]
    bsh = B // ncores
    cols = bsh * M
    ng = cols // GRP
    nq = cols // QCOLS

    rowmix = C.sum(axis=1)                      # [8], bias mix per module
    qb = mb2 @ aW1[D:, :]                       # [128]
    # rank-1 fold: (cur + y⊗s) @ aW1_top = cur @ aW1_top + s ⊗ qb
    y = np.linalg.solve(aW1[:D, :].T, qb)       # y @ aW1_top = qb

    consts = {
        "wm1": mW1.astype(BF16),
        "wmx": np.kron(np.eye(16, dtype=np.float32), C.T).astype(BF16),
        "wq": (mW2 @ aW1[D:, :]).astype(BF16),
        "wa1t": np.ascontiguousarray(aW1[:D, :]).astype(BF16),
        "wa2": aW2.astype(BF16),
        "mb1": np.ascontiguousarray(mb1.reshape(D, 1)),
        "ab1": np.ascontiguousarray(ab1.reshape(D, 1)),
        "ab2": np.ascontiguousarray(ab2.reshape(D, 1)),
    }

    in_maps = []
    for k in range(ncores):
        shard = ms[k * bsh : (k + 1) * bsh]
        xT = shard.reshape(cols, D).T                       # [128, cols]
        xb = np.ascontiguousarray(
            xT.reshape(D, nq, QCOLS).transpose(1, 0, 2)     # [nq, 128, 2048]
        ).astype(BF16)
        wk = w[k * bsh : (k + 1) * bsh]
        wflat = wk.reshape(cols)
        wcol = np.ascontiguousarray(wflat.reshape(4 * ng, D).T)
        s = (wk * rowmix[None, :]).reshape(cols)
        xaT = xT + y[:, None] * s[None, :]                  # [128, cols]
        xa = np.ascontiguousarray(
            xaT.reshape(D, nq, QCOLS).transpose(1, 0, 2)
        ).astype(BF16)
        in_maps.append({"xb": xb, "xa": xa, "wcol": wcol, **consts})
    return cols, in_maps


def gather_out(results, ab2, module_states=None, ncores=NCORES):
    ab2 = np.asarray(ab2, np.float32)
    outs = []
    for k in range(ncores):
        o = np.asarray(results[k]["out"]).astype(np.float32)
        nq = o.shape[0]
        cols = nq * QCOLS
        bsh = cols // M
        oT = o.transpose(1, 0, 2).reshape(D, cols)  # [128, cols]
        # device out = up1 + ab2 + up2; add x and the final step's ab2
        o = oT.T.reshape(bsh, M, D) + ab2[None, None, :]
        outs.append(o)
    out = np.concatenate(outs, 0)
    out += np.asarray(module_states, np.float32)
    return out.astype(np.float32)


def _run(inputs, trace=False):
    from concourse.bass_utils import run_bass_kernel_spmd

    cols, in_maps = host_prep(**inputs)
    nc = build_nc(cols)
    res = run_bass_kernel_spmd(nc, in_maps, list(range(NCORES)), trace=trace)
    out = gather_out(res.results, inputs["ab2"], inputs["module_states"])
    return out, res


def kernel(**inputs):
    out, _ = _run(inputs, trace=False)
    return out
